# revision 7
# baseline (speedup 1.0000x reference)
"""DiscreteOptionActor Trainium2 kernel (v3).

Computes, for each sample b, logits = MLP_{option[b]}(obs[b]) where each of the
16 options has its own 3-layer MLP (128 -> 256 -> 256 -> 18, ReLU).

Strategy (MoE routing):
  - Host groups samples by option (argsort); core k handles options 2k, 2k+1.
  - Per (core, option) the gathered rows are padded to PAD=4352, stored
    transposed (feature-major [128, PAD]) in fp16.
  - Weights packed into two byte tensors per option (w1+b1 first — it gates
    L1 — then w2+w3+b2), one DMA + one semaphore each.
  - Device: 3-layer MLP per option, fp16 matmuls (1 col/cycle at 2.4 GHz),
    fused bias+ReLU drains alternating ScalarE/VectorE. L3 (M=18) runs
    2-way col-tiled: even 512-blocks on PE col-group 0, odd blocks on
    col-group 1, so the two blocks' matmuls overlap in the array.
  - Host scatters results back and adds b3.
"""

import numpy as np

B, OBS, OPT, H1, H2, A = 65536, 128, 16, 256, 256, 18
NCORES = 8
OPC = OPT // NCORES  # options per core = 2
PAD = 4352

_CACHE = {}

# 1024-col drain granularity
PAIRS = [(0, 1024), (1024, 1024), (2048, 1024), (3072, 1024), (4096, 256)]
# xt DMA chunks (cols)
XCHUNKS = [(0, 512), (512, 512), (1024, 1024), (2048, 1024), (3072, 1280)]
N_WARM = 20
WARM_N = 128

# packed weight layouts (bytes per partition):
# w1b: w1 fp16 [256] bytes 0:512 ; b1 f32 [2] bytes 512:520
W1B_BYTES = 520
# w23: w2 fp16 [2,256] bytes 0:1024 ; w3 fp16 [2,18] bytes 1024:1096 ;
#      b2 f32 [2] bytes 1096:1104
W23_BYTES = 1104


def _halves(nb):
    out = []
    h = 0
    while h < nb:
        w = min(512, nb - h)
        out.append((h, w))
        h += w
    return out


def _chunk_hi(st, nb):
    """Index of the last xt chunk overlapping columns [st, st+nb)."""
    hi = 0
    for ci, (cst, cnb) in enumerate(XCHUNKS):
        if cst < st + nb:
            hi = ci
    return hi


def _build_v3():
    import concourse.bass as bass
    import concourse.bacc as bacc
    import concourse.mybir as mybir

    f32 = mybir.dt.float32
    f16 = mybir.dt.float16
    AF = mybir.ActivationFunctionType
    ALU = mybir.AluOpType

    nc = bacc.Bacc(None, target_bir_lowering=False, debug=False)
    xt = nc.declare_dram_parameter("xt", [OPC, OBS, PAD], f16, isOutput=False)
    w1b = nc.declare_dram_parameter("w1b", [OPC, 128, W1B_BYTES], mybir.dt.uint8,
                                    isOutput=False)
    w23 = nc.declare_dram_parameter("w23", [OPC, 128, W23_BYTES], mybir.dt.uint8,
                                    isOutput=False)
    out = nc.declare_dram_parameter("out", [OPC, A, PAD], f16, isOutput=True)

    # --- on-chip tensors ---
    xts = [nc.alloc_sbuf_tensor(f"xts{o}", [OBS, PAD], f16) for o in range(OPC)]
    w1s = [nc.alloc_sbuf_tensor(f"w1s{o}", [128, W1B_BYTES], mybir.dt.uint8)
           for o in range(OPC)]
    w23s = [nc.alloc_sbuf_tensor(f"w23s{o}", [128, W23_BYTES], mybir.dt.uint8)
            for o in range(OPC)]
    h1s = [[nc.alloc_sbuf_tensor(f"h1_{o}_{c}", [128, PAD], f16) for c in range(2)]
           for o in range(OPC)]
    h2s = [[nc.alloc_sbuf_tensor(f"h2_{o}_{m}", [128, PAD], f16) for m in range(2)]
           for o in range(OPC)]
    osbs = [nc.alloc_sbuf_tensor(f"osb{o}", [A, PAD], f16) for o in range(OPC)]
    dummy = nc.alloc_sbuf_tensor("warm_dummy", [128, WARM_N], f16)
    dummy_o = nc.alloc_sbuf_tensor("warm_dummy_o", [128, 1], f32)

    pss = [nc.alloc_psum_tensor(f"ps{s}", [128, 1024], f32) for s in range(4)]

    def w1_ap(o, c):
        return w1s[o].ap()[:, 0:512].bitcast(f16)[:, c * 128:(c + 1) * 128]

    def b1_ap(o, c):
        return w1s[o].ap()[:, 512:520].bitcast(f32)[:, c:c + 1]

    def w2_ap(o, k, m):
        base = k * 256 + m * 128
        return w23s[o].ap()[:, 0:1024].bitcast(f16)[:, base:base + 128]

    def w3_ap(o, k):
        return w23s[o].ap()[:, 1024:1096].bitcast(f16)[:, k * 18:(k + 1) * 18]

    def b2_ap(o, m):
        return w23s[o].ap()[:, 1096:1104].bitcast(f32)[:, m:m + 1]

    # --- semaphores ---
    ws = nc.alloc_semaphore("warm_sem")
    xsem = [[nc.alloc_semaphore(f"x{o}_{ci}") for ci in range(len(XCHUNKS))]
            for o in range(OPC)]
    w1sem = [nc.alloc_semaphore(f"w1t{o}") for o in range(OPC)]
    w23sem = [nc.alloc_semaphore(f"w23t{o}") for o in range(OPC)]
    fd = [nc.alloc_semaphore(f"fd{s}") for s in range(4)]
    prog = {}
    for o in range(OPC):
        for key in ("h1a", "h1v", "h2a", "h2v", "oa", "ov"):
            prog[(key, o)] = nc.alloc_semaphore(f"{key}{o}")
    odsem = nc.alloc_semaphore("od")

    # --- static schedule containers ---
    pe_ops = []
    act_ops = []
    dve_ops = []
    sync_ops = []
    gps_ops = []

    fill_count = [0, 0, 0, 0]
    slot_prev_drain = [None, None, None, None]
    fill_idx = 0
    prog_count = {k: 0 for k in prog}
    od_count = [0]

    pe_last_wait = {}

    def pe_wait(waits, sem, val):
        key = sem.name if hasattr(sem, "name") else id(sem)
        if pe_last_wait.get(key, -1) < val:
            waits.append((sem, val))
            pe_last_wait[key] = val

    def emit_fill(data_waits, mms, out_part, drains):
        """mms: (h, w, lhs_fn, rhs_fn, start, stop, mm_waits, tile_pos, ps_lo)
        drains: list of (engine, kind, dst_fn, bias_fn, psem_key, src_lo, src_hi, h0, w0)
        """
        nonlocal fill_idx
        s = fill_idx % 4
        fill_idx += 1
        waits = []
        if slot_prev_drain[s] is not None:
            for sem, cnt in slot_prev_drain[s]:
                pe_wait(waits, sem, cnt)
        for sem, val in data_waits:
            pe_wait(waits, sem, val)
        pe_ops.append((waits, s, mms, out_part, fd[s]))
        fill_count[s] += 1
        fd_thresh = fill_count[s]
        newprev = []
        for (drain_engine, kind, dst_fn, bias_fn, psem_key, src_lo, src_hi,
             h0, w0) in drains:
            sem = prog[psem_key]
            prog_count[psem_key] += 1
            cnt = prog_count[psem_key]
            op = ([(fd[s], fd_thresh)], kind, s, dst_fn, bias_fn, sem,
                  src_lo, src_hi, h0, w0)
            if drain_engine == "act":
                act_ops.append(op)
            else:
                dve_ops.append(op)
            newprev.append((sem, cnt))
        slot_prev_drain[s] = newprev
        return {k: prog_count[k] for k in prog}

    # --- input DMA schedule ---
    def xdma(o, ci):
        cst, cnb = XCHUNKS[ci]
        return ("dma", [],
                (lambda o=o, cst=cst, cnb=cnb: xts[o].ap()[:, cst:cst + cnb]),
                (lambda o=o, cst=cst, cnb=cnb: xt[o][:, cst:cst + cnb]),
                xsem[o][ci], 16)

    def wdma(o, which):
        if which == 0:
            return ("dma", [], (lambda o=o: w1s[o].ap()[:]),
                    (lambda o=o: w1b[o]), w1sem[o], 16)
        return ("dma", [], (lambda o=o: w23s[o].ap()[:]),
                (lambda o=o: w23[o]), w23sem[o], 16)

    # gpsimd dispatches its body earliest (~6.2us) -> critical w1b0 + spare
    # chunks; sync (~6.9us) takes x0c0 first then the rest.
    gps_ops.extend([wdma(0, 0), xdma(0, 1), xdma(0, 3), xdma(0, 4),
                    xdma(1, 0), xdma(1, 1), xdma(1, 2)])
    sync_ops.extend([xdma(0, 0), wdma(0, 1), xdma(0, 2), wdma(1, 0),
                     wdma(1, 1), xdma(1, 3), xdma(1, 4)])

    di = 0
    l1_thr = {}
    l2_thr = {}

    def emit_l1(o, p):
        nonlocal di
        st, nb = PAIRS[p]
        mms = []
        drains = []
        for c in range(2):
            for h, w in _halves(nb):
                hi = _chunk_hi(st + h, w)
                mms.append((
                    h, w,
                    (lambda o=o, c=c: w1_ap(o, c)),
                    (lambda o=o, st=st, h=h, w=w: xts[o].ap()[:, st + h:st + h + w]),
                    True, True, [(xsem[o][hi], 16)], None, 0,
                ))
            eng = "act" if di % 2 == 0 else "dve"
            drains.append((eng, "relu",
                           (lambda o=o, c=c, st=st, nb=nb: h1s[o][c].ap()[:, st:st + nb]),
                           (lambda o=o, c=c: b1_ap(o, c)),
                           ("h1a" if eng == "act" else "h1v", o), 0, 128, 0, nb))
            di += 1
            # both c-chunks of one pair share a psum slot? no: separate fills
            pc = emit_fill([(w1sem[o], 16)], mms, 128, drains)
            mms = []
            drains = []
        l1_thr[(o, p)] = (pc[("h1a", o)], pc[("h1v", o)])

    def emit_l2(o, p):
        nonlocal di
        st, nb = PAIRS[p]
        na, nv = l1_thr[(o, p)]
        pc = None
        for m in range(2):
            data_waits = [(w23sem[o], 16)]
            if na:
                data_waits.append((prog[("h1a", o)], na))
            if nv:
                data_waits.append((prog[("h1v", o)], nv))
            mms = []
            for h, w in _halves(nb):
                for k in range(2):
                    mms.append((
                        h, w,
                        (lambda o=o, k=k, m=m: w2_ap(o, k, m)),
                        (lambda o=o, k=k, st=st, h=h, w=w: h1s[o][k].ap()[:, st + h:st + h + w]),
                        k == 0, k == 1, None, None, 0,
                    ))
            eng = "act" if di % 2 == 0 else "dve"
            drains = [(eng, "relu",
                       (lambda o=o, m=m, st=st, nb=nb: h2s[o][m].ap()[:, st:st + nb]),
                       (lambda o=o, m=m: b2_ap(o, m)),
                       ("h2a" if eng == "act" else "h2v", o), 0, 128, 0, nb)]
            di += 1
            pc = emit_fill(data_waits, mms, 128, drains)
        l2_thr[(o, p)] = (pc[("h2a", o)], pc[("h2v", o)])

    def emit_l3(o, p):
        nonlocal di
        st, nb = PAIRS[p]
        na, nv = l2_thr[(o, p)]
        data_waits = [(w23sem[o], 16)]
        if na:
            data_waits.append((prog[("h2a", o)], na))
        if nv:
            data_waits.append((prog[("h2v", o)], nv))
        mms = []
        drains = []
        # even 512-block -> col group 0 (psum partitions 0:18, ACT drain);
        # odd 512-block -> col group 1 (psum partitions 32:50, DVE drain)
        for bi, (h, w) in enumerate(_halves(nb)):
            grp = bi % 2
            plo = 32 * grp
            tp = (0, plo)
            for k in range(2):
                mms.append((
                    h, w,
                    (lambda o=o, k=k: w3_ap(o, k)),
                    (lambda o=o, k=k, st=st, h=h, w=w: h2s[o][k].ap()[:, st + h:st + h + w]),
                    k == 0, k == 1, None, tp, plo,
                ))
            eng = "act" if grp == 0 else "dve"
            drains.append((eng, "copy",
                           (lambda o=o, st=st, h=h, w=w: osbs[o].ap()[:, st + h:st + h + w]),
                           None,
                           ("oa" if eng == "act" else "ov", o),
                           plo, plo + A, h, w))
        di += 1
        pc = emit_fill(data_waits, mms, A, drains)
        dma_waits = []
        if pc[("oa", o)]:
            dma_waits.append((prog[("oa", o)], pc[("oa", o)]))
        if pc[("ov", o)] and nb == 1024:
            dma_waits.append((prog[("ov", o)], pc[("ov", o)]))
        od_count[0] += 1
        dma_op = ("dma", dma_waits,
                  (lambda o=o, st=st, nb=nb: out[o][:, st:st + nb]),
                  (lambda o=o, st=st, nb=nb: osbs[o].ap()[:, st:st + nb]),
                  odsem, 16)
        sync_ops.append(dma_op)

    # global software pipeline: L1 two pair-groups ahead; L3 trails by one
    l1q = [(o, p) for o in range(OPC) for p in range(len(PAIRS))]
    l2q = list(l1q)
    l3q = []
    emit_l1(*l1q.pop(0))
    emit_l1(*l1q.pop(0))
    for (o, p) in l2q:
        if l1q:
            emit_l1(*l1q.pop(0))
        emit_l2(o, p)
        l3q.append((o, p))
        if len(l3q) > 1:
            emit_l3(*l3q.pop(0))
    while l3q:
        emit_l3(*l3q.pop(0))

    n_od = od_count[0]

    # --- emit engine programs ---
    with nc.Block() as block:

        @block.gpsimd
        def _(eng):
            nc.gpsimd.memset(dummy.ap()[:], 0.0).then_inc(ws, 1)
            for op in gps_ops:
                kind, waits, dst_fn, src_fn, sem, val = op
                for wsem_, wval in waits:
                    eng.wait_ge(wsem_, wval)
                eng.dma_start(out=dst_fn(), in_=src_fn()).then_inc(sem, val)

        @block.sync
        def _(eng):
            for op in sync_ops:
                kind, waits, dst_fn, src_fn, sem, val = op
                for wsem_, wval in waits:
                    eng.wait_ge(wsem_, wval)
                eng.dma_start(out=dst_fn(), in_=src_fn()).then_inc(sem, val)
            eng.wait_ge(odsem, 16 * n_od)

        @block.tensor
        def _(eng):
            eng.wait_ge(ws, 1)
            for _i in range(N_WARM):
                nc.tensor.matmul(
                    pss[0].ap()[:128, :WARM_N], dummy.ap()[:, :], dummy.ap()[:, :],
                    start=True, stop=True,
                )
            mm_seen = {}
            for waits, s, mms, out_part, fdsem in pe_ops:
                for wsem_, wval in waits:
                    eng.wait_ge(wsem_, wval)
                for j, (h, w, lhs_fn, rhs_fn, stt, stp, mwaits, tp, plo) in enumerate(mms):
                    if mwaits:
                        for wsem_, wval in mwaits:
                            key = wsem_.name if hasattr(wsem_, "name") else id(wsem_)
                            if mm_seen.get(key, -1) < wval:
                                eng.wait_ge(wsem_, wval)
                                mm_seen[key] = wval
                    kw = {}
                    if tp is not None:
                        kw["tile_position"] = tp
                    inst = nc.tensor.matmul(
                        pss[s].ap()[plo:plo + out_part, h:h + w],
                        lhs_fn(), rhs_fn(), start=stt, stop=stp, **kw,
                    )
                    if j == len(mms) - 1:
                        inst.then_inc(fdsem, 1)

        @block.scalar
        def _(eng):
            eng.wait_ge(ws, 1)
            nc.scalar.activation(dummy_o.ap()[:], dummy.ap()[:, 0:1], AF.Relu, bias=0.0)
            for waits, kind, s, dst_fn, bias_fn, sem, lo, hi, h0, w0 in act_ops:
                for wsem_, wval in waits:
                    eng.wait_ge(wsem_, wval)
                dst = dst_fn()
                src = pss[s].ap()[lo:hi, h0:h0 + w0]
                if kind == "relu":
                    inst = nc.scalar.activation(dst, src, AF.Relu, bias=bias_fn())
                else:
                    inst = nc.scalar.activation(dst, src, AF.Copy)
                inst.then_inc(sem, 1)

        @block.vector
        def _(eng):
            for waits, kind, s, dst_fn, bias_fn, sem, lo, hi, h0, w0 in dve_ops:
                for wsem_, wval in waits:
                    eng.wait_ge(wsem_, wval)
                dst = dst_fn()
                src = pss[s].ap()[lo:hi, h0:h0 + w0]
                if kind == "relu":
                    inst = nc.vector.tensor_scalar(
                        dst, src, bias_fn(), 0.0, ALU.add, ALU.max
                    )
                else:
                    inst = nc.vector.tensor_copy(dst, src)
                inst.then_inc(sem, 1)

    nc.compile()
    return nc


def _get_program():
    if "nc" not in _CACHE:
        _CACHE["nc"] = _build_v3()
    return _CACHE["nc"]


def _prep(inputs):
    obs = np.ascontiguousarray(np.asarray(inputs["obs"], dtype=np.float32))
    option = np.asarray(inputs["option"]).astype(np.int64, copy=False)
    W1 = np.asarray(inputs["W1"], dtype=np.float32)
    b1 = np.asarray(inputs["b1"], dtype=np.float32)
    W2 = np.asarray(inputs["W2"], dtype=np.float32)
    b2 = np.asarray(inputs["b2"], dtype=np.float32)
    W3 = np.asarray(inputs["W3"], dtype=np.float32)
    b3 = np.asarray(inputs["b3"], dtype=np.float32)

    order = np.argsort(option, kind="stable")
    sorted_opt = option[order]
    starts = np.searchsorted(sorted_opt, np.arange(OPT + 1))
    idx_per_opt = [order[starts[o]: starts[o + 1]] for o in range(OPT)]

    in_maps = []
    for core in range(NCORES):
        xtc = np.zeros((OPC, OBS, PAD), np.float16)
        w1bc = np.zeros((OPC, 128, W1B_BYTES), np.uint8)
        w23c = np.zeros((OPC, 128, W23_BYTES), np.uint8)
        for lo in range(OPC):
            o = core * OPC + lo
            idx = idx_per_opt[o][:PAD]
            xtc[lo, :, : len(idx)] = obs[idx].T
            w1p = np.ascontiguousarray(W1[o].astype(np.float16))  # [128, 256]
            b1p = np.ascontiguousarray(b1[o].reshape(2, 128).T.astype(np.float32))
            w1bc[lo] = np.concatenate(
                [w1p.view(np.uint8), b1p.view(np.uint8)], axis=1)
            w2p = np.ascontiguousarray(
                W2[o].reshape(2, 128, H2).transpose(1, 0, 2).astype(np.float16)
            ).reshape(128, -1)                                    # [128, 512]
            w3p = np.ascontiguousarray(
                W3[o].reshape(2, 128, A).transpose(1, 0, 2).astype(np.float16)
            ).reshape(128, -1)                                    # [128, 36]
            b2p = np.ascontiguousarray(b2[o].reshape(2, 128).T.astype(np.float32))
            w23c[lo] = np.concatenate(
                [w2p.view(np.uint8), w3p.view(np.uint8), b2p.view(np.uint8)],
                axis=1)
        in_maps.append({"xt": xtc, "w1b": w1bc, "w23": w23c})
    host = dict(obs=obs, W1=W1, b1=b1, W2=W2, b2=b2, W3=W3, b3=b3)
    return in_maps, idx_per_opt, host


def _unshard(results, idx_per_opt, host):
    out_full = np.empty((B, 1, A), np.float32)
    for core in range(NCORES):
        res = results[core]["out"]  # [OPC, A, PAD]
        for lo in range(OPC):
            o = core * OPC + lo
            idx = idx_per_opt[o]
            n = min(len(idx), PAD)
            out_full[idx[:n], 0, :] = res[lo, :, :n].T + host["b3"][o]
            if len(idx) > n:  # overflow beyond PAD: compute on host (rare/never)
                rows = host["obs"][idx[n:]]
                h = np.maximum(rows @ host["W1"][o] + host["b1"][o], 0.0)
                h = np.maximum(h @ host["W2"][o] + host["b2"][o], 0.0)
                out_full[idx[n:], 0, :] = h @ host["W3"][o] + host["b3"][o]
    return out_full


def run(inputs, trace=False, **spmd_kwargs):
    """Run the kernel; returns (output, BassKernelResults)."""
    from concourse.bass_utils import run_bass_kernel_spmd

    in_maps, idx_per_opt, host = _prep(inputs)
    nc = _get_program()
    try:
        br = run_bass_kernel_spmd(
            nc, in_maps, list(range(NCORES)), trace=trace, **spmd_kwargs
        )
    except Exception:
        _CACHE.clear()
        nc = _get_program()
        br = run_bass_kernel_spmd(
            nc, in_maps, list(range(NCORES)), trace=trace, **spmd_kwargs
        )
    return _unshard(br.results, idx_per_opt, host), br


def kernel(**inputs):
    out, _ = run(inputs)
    return out


# revision 11
# speedup vs baseline: 1.0671x; 1.0671x over previous
"""DiscreteOptionActor Trainium2 kernel (v3).

Computes, for each sample b, logits = MLP_{option[b]}(obs[b]) where each of the
16 options has its own 3-layer MLP (128 -> 256 -> 256 -> 18, ReLU).

Strategy (MoE routing):
  - Host groups samples by option (argsort); core k handles options 2k, 2k+1.
  - Per (core, option) the gathered rows are padded to PAD=4352, stored
    transposed (feature-major [128, PAD]) in fp16.
  - Weights packed into two byte tensors per option (w1+b1 first — it gates
    L1 — then w2+w3+b2), one DMA + one semaphore each.
  - Device: 3-layer MLP per option, fp16 matmuls (1 col/cycle at 2.4 GHz),
    fused bias+ReLU drains alternating ScalarE/VectorE. L3 (M=18) runs
    2-way col-tiled: even 512-blocks on PE col-group 0, odd blocks on
    col-group 1, so the two blocks' matmuls overlap in the array.
  - Host scatters results back and adds b3.
"""

import numpy as np

B, OBS, OPT, H1, H2, A = 65536, 128, 16, 256, 256, 18
NCORES = 8
OPC = OPT // NCORES  # options per core = 2
PAD = 4352

_CACHE = {}

# 1024-col drain granularity
PAIRS = [(0, 1024), (1024, 1024), (2048, 1024), (3072, 1024), (4096, 256)]
# xt DMA chunks (cols)
XCHUNKS = [(0, 512), (512, 512), (1024, 1024), (2048, 1024), (3072, 1280)]
N_WARM = 12
WARM_N = 128

# packed weight layouts (bytes per partition):
# w1b: w1 fp16 [256] bytes 0:512 ; b1 f32 [2] bytes 512:520
W1B_BYTES = 520
# w23: w2 fp16 [2,256] bytes 0:1024 ; w3 fp16 [2,18] bytes 1024:1096 ;
#      b2 f32 [2] bytes 1096:1104
W23_BYTES = 1104


def _halves(nb):
    out = []
    h = 0
    while h < nb:
        w = min(512, nb - h)
        out.append((h, w))
        h += w
    return out


def _chunk_hi(st, nb):
    """Index of the last xt chunk overlapping columns [st, st+nb)."""
    hi = 0
    for ci, (cst, cnb) in enumerate(XCHUNKS):
        if cst < st + nb:
            hi = ci
    return hi


def _build_v3():
    import concourse.bass as bass
    import concourse.bacc as bacc
    import concourse.mybir as mybir

    f32 = mybir.dt.float32
    f16 = mybir.dt.float16
    AF = mybir.ActivationFunctionType
    ALU = mybir.AluOpType

    nc = bacc.Bacc(None, target_bir_lowering=False, debug=False)
    xt = nc.declare_dram_parameter("xt", [OPC, OBS, PAD], f16, isOutput=False)
    w1b = nc.declare_dram_parameter("w1b", [OPC, 128, W1B_BYTES], mybir.dt.uint8,
                                    isOutput=False)
    w23 = nc.declare_dram_parameter("w23", [OPC, 128, W23_BYTES], mybir.dt.uint8,
                                    isOutput=False)
    out = nc.declare_dram_parameter("out", [OPC, A, PAD], f16, isOutput=True)

    # --- on-chip tensors ---
    xts = [nc.alloc_sbuf_tensor(f"xts{o}", [OBS, PAD], f16) for o in range(OPC)]
    w1s = [nc.alloc_sbuf_tensor(f"w1s{o}", [128, W1B_BYTES], mybir.dt.uint8)
           for o in range(OPC)]
    w23s = [nc.alloc_sbuf_tensor(f"w23s{o}", [128, W23_BYTES], mybir.dt.uint8)
            for o in range(OPC)]
    h1s = [[nc.alloc_sbuf_tensor(f"h1_{o}_{c}", [128, PAD], f16) for c in range(2)]
           for o in range(OPC)]
    h2s = [[nc.alloc_sbuf_tensor(f"h2_{o}_{m}", [128, PAD], f16) for m in range(2)]
           for o in range(OPC)]
    osbs = [nc.alloc_sbuf_tensor(f"osb{o}", [A, PAD], f16) for o in range(OPC)]
    dummy = nc.alloc_sbuf_tensor("warm_dummy", [128, WARM_N], f16)
    dummy_o = nc.alloc_sbuf_tensor("warm_dummy_o", [128, 1], f32)

    pss = [nc.alloc_psum_tensor(f"ps{s}", [128, 1024], f32) for s in range(4)]

    def w1_ap(o, c):
        return w1s[o].ap()[:, 0:512].bitcast(f16)[:, c * 128:(c + 1) * 128]

    def b1_ap(o, c):
        return w1s[o].ap()[:, 512:520].bitcast(f32)[:, c:c + 1]

    def w2_ap(o, k, m):
        base = k * 256 + m * 128
        return w23s[o].ap()[:, 0:1024].bitcast(f16)[:, base:base + 128]

    def w3_ap(o, k):
        return w23s[o].ap()[:, 1024:1096].bitcast(f16)[:, k * 18:(k + 1) * 18]

    def b2_ap(o, m):
        return w23s[o].ap()[:, 1096:1104].bitcast(f32)[:, m:m + 1]

    # --- semaphores ---
    ws = nc.alloc_semaphore("warm_sem")
    xsem = [[nc.alloc_semaphore(f"x{o}_{ci}") for ci in range(len(XCHUNKS))]
            for o in range(OPC)]
    w1sem = [nc.alloc_semaphore(f"w1t{o}") for o in range(OPC)]
    w23sem = [nc.alloc_semaphore(f"w23t{o}") for o in range(OPC)]
    fd = [nc.alloc_semaphore(f"fd{s}") for s in range(4)]
    prog = {}
    for o in range(OPC):
        for key in ("h1a", "h1v", "h2a", "h2v", "oa", "ov"):
            prog[(key, o)] = nc.alloc_semaphore(f"{key}{o}")
    odsem = nc.alloc_semaphore("od")

    # --- static schedule containers ---
    pe_ops = []
    act_ops = []
    dve_ops = []
    sync_ops = []
    gps_ops = []

    fill_count = [0, 0, 0, 0]
    slot_prev_drain = [None, None, None, None]
    fill_idx = 0
    prog_count = {k: 0 for k in prog}
    od_count = [0]

    pe_last_wait = {}

    def pe_wait(waits, sem, val):
        key = sem.name if hasattr(sem, "name") else id(sem)
        if pe_last_wait.get(key, -1) < val:
            waits.append((sem, val))
            pe_last_wait[key] = val

    def emit_fill(data_waits, mms, out_part, drains):
        """mms: (h, w, lhs_fn, rhs_fn, start, stop, mm_waits, tile_pos, ps_lo)
        drains: list of (engine, kind, dst_fn, bias_fn, psem_key, src_lo, src_hi, h0, w0)
        """
        nonlocal fill_idx
        s = fill_idx % 4
        fill_idx += 1
        waits = []
        if slot_prev_drain[s] is not None:
            for sem, cnt in slot_prev_drain[s]:
                pe_wait(waits, sem, cnt)
        for sem, val in data_waits:
            pe_wait(waits, sem, val)
        pe_ops.append((waits, s, mms, out_part, fd[s]))
        fill_count[s] += 1
        fd_thresh = fill_count[s]
        newprev = []
        for (drain_engine, kind, dst_fn, bias_fn, psem_key, src_lo, src_hi,
             h0, w0) in drains:
            sem = prog[psem_key]
            prog_count[psem_key] += 1
            cnt = prog_count[psem_key]
            op = ([(fd[s], fd_thresh)], kind, s, dst_fn, bias_fn, sem,
                  src_lo, src_hi, h0, w0)
            if drain_engine == "act":
                act_ops.append(op)
            else:
                dve_ops.append(op)
            newprev.append((sem, cnt))
        slot_prev_drain[s] = newprev
        return {k: prog_count[k] for k in prog}

    # --- input DMA schedule ---
    def xdma(o, ci):
        cst, cnb = XCHUNKS[ci]
        return ("dma", [],
                (lambda o=o, cst=cst, cnb=cnb: xts[o].ap()[:, cst:cst + cnb]),
                (lambda o=o, cst=cst, cnb=cnb: xt[o][:, cst:cst + cnb]),
                xsem[o][ci], 16)

    def wdma(o, which):
        if which == 0:
            return ("dma", [], (lambda o=o: w1s[o].ap()[:]),
                    (lambda o=o: w1b[o]), w1sem[o], 16)
        return ("dma", [], (lambda o=o: w23s[o].ap()[:]),
                (lambda o=o: w23[o]), w23sem[o], 16)

    # Three parallel issue queues for the startup-critical transfers:
    # scalar (idle until ~11.5us) takes x0c0, sync takes w1b0 then the o0
    # stream, gpsimd (slow ~1.1us body entry) takes the late chunks.
    scalar_ops = [xdma(0, 0)]
    sync_ops.extend([wdma(0, 0), xdma(0, 1), wdma(0, 1), xdma(0, 2),
                     wdma(1, 0), wdma(1, 1)])
    gps_ops.extend([xdma(0, 3), xdma(0, 4), xdma(1, 0), xdma(1, 1),
                    xdma(1, 2), xdma(1, 3), xdma(1, 4)])

    di = 0
    l1_thr = {}
    l2_thr = {}

    def emit_l1(o, p):
        nonlocal di
        st, nb = PAIRS[p]
        mms = []
        drains = []
        for c in range(2):
            for h, w in _halves(nb):
                hi = _chunk_hi(st + h, w)
                mms.append((
                    h, w,
                    (lambda o=o, c=c: w1_ap(o, c)),
                    (lambda o=o, st=st, h=h, w=w: xts[o].ap()[:, st + h:st + h + w]),
                    True, True, [(xsem[o][hi], 16)], None, 0,
                ))
            eng = "act" if di % 2 == 0 else "dve"
            drains.append((eng, "relu",
                           (lambda o=o, c=c, st=st, nb=nb: h1s[o][c].ap()[:, st:st + nb]),
                           (lambda o=o, c=c: b1_ap(o, c)),
                           ("h1a" if eng == "act" else "h1v", o), 0, 128, 0, nb))
            di += 1
            # both c-chunks of one pair share a psum slot? no: separate fills
            pc = emit_fill([(w1sem[o], 16)], mms, 128, drains)
            mms = []
            drains = []
        l1_thr[(o, p)] = (pc[("h1a", o)], pc[("h1v", o)])

    def emit_l2(o, p):
        nonlocal di
        st, nb = PAIRS[p]
        na, nv = l1_thr[(o, p)]
        pc = None
        for m in range(2):
            data_waits = [(w23sem[o], 16)]
            if na:
                data_waits.append((prog[("h1a", o)], na))
            if nv:
                data_waits.append((prog[("h1v", o)], nv))
            mms = []
            for h, w in _halves(nb):
                for k in range(2):
                    mms.append((
                        h, w,
                        (lambda o=o, k=k, m=m: w2_ap(o, k, m)),
                        (lambda o=o, k=k, st=st, h=h, w=w: h1s[o][k].ap()[:, st + h:st + h + w]),
                        k == 0, k == 1, None, None, 0,
                    ))
            eng = "act" if di % 2 == 0 else "dve"
            drains = [(eng, "relu",
                       (lambda o=o, m=m, st=st, nb=nb: h2s[o][m].ap()[:, st:st + nb]),
                       (lambda o=o, m=m: b2_ap(o, m)),
                       ("h2a" if eng == "act" else "h2v", o), 0, 128, 0, nb)]
            di += 1
            pc = emit_fill(data_waits, mms, 128, drains)
        l2_thr[(o, p)] = (pc[("h2a", o)], pc[("h2v", o)])

    def emit_l3(o, p):
        nonlocal di
        st, nb = PAIRS[p]
        na, nv = l2_thr[(o, p)]
        data_waits = [(w23sem[o], 16)]
        if na:
            data_waits.append((prog[("h2a", o)], na))
        if nv:
            data_waits.append((prog[("h2v", o)], nv))
        mms = []
        for h, w in _halves(nb):
            for k in range(2):
                mms.append((
                    h, w,
                    (lambda o=o, k=k: w3_ap(o, k)),
                    (lambda o=o, k=k, st=st, h=h, w=w: h2s[o][k].ap()[:, st + h:st + h + w]),
                    k == 0, k == 1, None, None, 0,
                ))
        eng = "act" if (nb < 1024 or di % 2 == 0) else "dve"
        drains = [(eng, "copy",
                   (lambda o=o, st=st, nb=nb: osbs[o].ap()[:, st:st + nb]),
                   None,
                   ("oa" if eng == "act" else "ov", o), 0, A, 0, nb)]
        di += 1
        pc = emit_fill(data_waits, mms, A, drains)
        dma_waits = [(prog[("oa" if eng == "act" else "ov", o)],
                      pc[("oa" if eng == "act" else "ov", o)])]
        od_count[0] += 1
        dma_op = ("dma", dma_waits,
                  (lambda o=o, st=st, nb=nb: out[o][:, st:st + nb]),
                  (lambda o=o, st=st, nb=nb: osbs[o].ap()[:, st:st + nb]),
                  odsem, 16)
        sync_ops.append(dma_op)

    # global software pipeline: L1 two pair-groups ahead; L3 trails by one
    l1q = [(o, p) for o in range(OPC) for p in range(len(PAIRS))]
    l2q = list(l1q)
    l3q = []
    emit_l1(*l1q.pop(0))
    emit_l1(*l1q.pop(0))
    for (o, p) in l2q:
        if l1q:
            emit_l1(*l1q.pop(0))
        emit_l2(o, p)
        l3q.append((o, p))
        if len(l3q) > 1:
            emit_l3(*l3q.pop(0))
    while l3q:
        emit_l3(*l3q.pop(0))

    n_od = od_count[0]

    # --- emit engine programs ---
    with nc.Block() as block:

        @block.gpsimd
        def _(eng):
            nc.gpsimd.memset(dummy.ap()[:], 0.0).then_inc(ws, 1)
            for op in gps_ops:
                kind, waits, dst_fn, src_fn, sem, val = op
                for wsem_, wval in waits:
                    eng.wait_ge(wsem_, wval)
                eng.dma_start(out=dst_fn(), in_=src_fn()).then_inc(sem, val)

        @block.sync
        def _(eng):
            for op in sync_ops:
                kind, waits, dst_fn, src_fn, sem, val = op
                for wsem_, wval in waits:
                    eng.wait_ge(wsem_, wval)
                eng.dma_start(out=dst_fn(), in_=src_fn()).then_inc(sem, val)
            eng.wait_ge(odsem, 16 * n_od)

        @block.tensor
        def _(eng):
            eng.wait_ge(ws, 1)
            for _i in range(N_WARM):
                nc.tensor.matmul(
                    pss[0].ap()[:128, :WARM_N], dummy.ap()[:, :], dummy.ap()[:, :],
                    start=True, stop=True,
                )
            mm_seen = {}
            for waits, s, mms, out_part, fdsem in pe_ops:
                for wsem_, wval in waits:
                    eng.wait_ge(wsem_, wval)
                for j, (h, w, lhs_fn, rhs_fn, stt, stp, mwaits, tp, plo) in enumerate(mms):
                    if mwaits:
                        for wsem_, wval in mwaits:
                            key = wsem_.name if hasattr(wsem_, "name") else id(wsem_)
                            if mm_seen.get(key, -1) < wval:
                                eng.wait_ge(wsem_, wval)
                                mm_seen[key] = wval
                    kw = {}
                    if tp is not None:
                        kw["tile_position"] = tp
                    inst = nc.tensor.matmul(
                        pss[s].ap()[plo:plo + out_part, h:h + w],
                        lhs_fn(), rhs_fn(), start=stt, stop=stp, **kw,
                    )
                    if j == len(mms) - 1:
                        inst.then_inc(fdsem, 1)

        @block.scalar
        def _(eng):
            for op in scalar_ops:
                kind, waits, dst_fn, src_fn, sem, val = op
                eng.dma_start(out=dst_fn(), in_=src_fn()).then_inc(sem, val)
            eng.wait_ge(ws, 1)
            nc.scalar.activation(dummy_o.ap()[:], dummy.ap()[:, 0:1], AF.Relu, bias=0.0)
            for waits, kind, s, dst_fn, bias_fn, sem, lo, hi, h0, w0 in act_ops:
                for wsem_, wval in waits:
                    eng.wait_ge(wsem_, wval)
                dst = dst_fn()
                src = pss[s].ap()[lo:hi, h0:h0 + w0]
                if kind == "relu":
                    inst = nc.scalar.activation(dst, src, AF.Relu, bias=bias_fn())
                else:
                    inst = nc.scalar.activation(dst, src, AF.Copy)
                inst.then_inc(sem, 1)

        @block.vector
        def _(eng):
            for waits, kind, s, dst_fn, bias_fn, sem, lo, hi, h0, w0 in dve_ops:
                for wsem_, wval in waits:
                    eng.wait_ge(wsem_, wval)
                dst = dst_fn()
                src = pss[s].ap()[lo:hi, h0:h0 + w0]
                if kind == "relu":
                    inst = nc.vector.tensor_scalar(
                        dst, src, bias_fn(), 0.0, ALU.add, ALU.max
                    )
                else:
                    inst = nc.vector.tensor_copy(dst, src)
                inst.then_inc(sem, 1)

    nc.compile()
    return nc


def _get_program():
    if "nc" not in _CACHE:
        _CACHE["nc"] = _build_v3()
    return _CACHE["nc"]


def _prep(inputs):
    obs = np.ascontiguousarray(np.asarray(inputs["obs"], dtype=np.float32))
    option = np.asarray(inputs["option"]).astype(np.int64, copy=False)
    W1 = np.asarray(inputs["W1"], dtype=np.float32)
    b1 = np.asarray(inputs["b1"], dtype=np.float32)
    W2 = np.asarray(inputs["W2"], dtype=np.float32)
    b2 = np.asarray(inputs["b2"], dtype=np.float32)
    W3 = np.asarray(inputs["W3"], dtype=np.float32)
    b3 = np.asarray(inputs["b3"], dtype=np.float32)

    order = np.argsort(option, kind="stable")
    sorted_opt = option[order]
    starts = np.searchsorted(sorted_opt, np.arange(OPT + 1))
    idx_per_opt = [order[starts[o]: starts[o + 1]] for o in range(OPT)]

    in_maps = []
    for core in range(NCORES):
        xtc = np.zeros((OPC, OBS, PAD), np.float16)
        w1bc = np.zeros((OPC, 128, W1B_BYTES), np.uint8)
        w23c = np.zeros((OPC, 128, W23_BYTES), np.uint8)
        for lo in range(OPC):
            o = core * OPC + lo
            idx = idx_per_opt[o][:PAD]
            xtc[lo, :, : len(idx)] = obs[idx].T
            w1p = np.ascontiguousarray(W1[o].astype(np.float16))  # [128, 256]
            b1p = np.ascontiguousarray(b1[o].reshape(2, 128).T.astype(np.float32))
            w1bc[lo] = np.concatenate(
                [w1p.view(np.uint8), b1p.view(np.uint8)], axis=1)
            w2p = np.ascontiguousarray(
                W2[o].reshape(2, 128, H2).transpose(1, 0, 2).astype(np.float16)
            ).reshape(128, -1)                                    # [128, 512]
            w3p = np.ascontiguousarray(
                W3[o].reshape(2, 128, A).transpose(1, 0, 2).astype(np.float16)
            ).reshape(128, -1)                                    # [128, 36]
            b2p = np.ascontiguousarray(b2[o].reshape(2, 128).T.astype(np.float32))
            w23c[lo] = np.concatenate(
                [w2p.view(np.uint8), w3p.view(np.uint8), b2p.view(np.uint8)],
                axis=1)
        in_maps.append({"xt": xtc, "w1b": w1bc, "w23": w23c})
    host = dict(obs=obs, W1=W1, b1=b1, W2=W2, b2=b2, W3=W3, b3=b3)
    return in_maps, idx_per_opt, host


def _unshard(results, idx_per_opt, host):
    out_full = np.empty((B, 1, A), np.float32)
    for core in range(NCORES):
        res = results[core]["out"]  # [OPC, A, PAD]
        for lo in range(OPC):
            o = core * OPC + lo
            idx = idx_per_opt[o]
            n = min(len(idx), PAD)
            out_full[idx[:n], 0, :] = res[lo, :, :n].T + host["b3"][o]
            if len(idx) > n:  # overflow beyond PAD: compute on host (rare/never)
                rows = host["obs"][idx[n:]]
                h = np.maximum(rows @ host["W1"][o] + host["b1"][o], 0.0)
                h = np.maximum(h @ host["W2"][o] + host["b2"][o], 0.0)
                out_full[idx[n:], 0, :] = h @ host["W3"][o] + host["b3"][o]
    return out_full


def run(inputs, trace=False, **spmd_kwargs):
    """Run the kernel; returns (output, BassKernelResults)."""
    from concourse.bass_utils import run_bass_kernel_spmd

    in_maps, idx_per_opt, host = _prep(inputs)
    nc = _get_program()
    try:
        br = run_bass_kernel_spmd(
            nc, in_maps, list(range(NCORES)), trace=trace, **spmd_kwargs
        )
    except Exception:
        _CACHE.clear()
        nc = _get_program()
        br = run_bass_kernel_spmd(
            nc, in_maps, list(range(NCORES)), trace=trace, **spmd_kwargs
        )
    return _unshard(br.results, idx_per_opt, host), br


def kernel(**inputs):
    out, _ = run(inputs)
    return out


# revision 14
# speedup vs baseline: 1.0780x; 1.0102x over previous
"""DiscreteOptionActor Trainium2 kernel (v3).

Computes, for each sample b, logits = MLP_{option[b]}(obs[b]) where each of the
16 options has its own 3-layer MLP (128 -> 256 -> 256 -> 18, ReLU).

Strategy (MoE routing):
  - Host groups samples by option (argsort); core k handles options 2k, 2k+1.
  - Per (core, option) the gathered rows are padded to PAD=4352, stored
    transposed (feature-major [128, PAD]) in fp16.
  - Weights packed into two byte tensors per option (w1+b1 first — it gates
    L1 — then w2+w3+b2), one DMA + one semaphore each.
  - Device: 3-layer MLP per option, fp16 matmuls (1 col/cycle at 2.4 GHz),
    fused bias+ReLU drains alternating ScalarE/VectorE. L3 (M=18) runs
    2-way col-tiled: even 512-blocks on PE col-group 0, odd blocks on
    col-group 1, so the two blocks' matmuls overlap in the array.
  - Host scatters results back and adds b3.
"""

import numpy as np

B, OBS, OPT, H1, H2, A = 65536, 128, 16, 256, 256, 18
NCORES = 8
OPC = OPT // NCORES  # options per core = 2
PAD = 4352

_CACHE = {}

# 1024-col drain granularity
PAIRS = [(0, 1024), (1024, 1024), (2048, 1024), (3072, 1024), (4096, 256)]
# xt DMA chunks (cols)
XCHUNKS = [(0, 512), (512, 512), (1024, 1024), (2048, 1024), (3072, 1280)]
N_WARM = 20
WARM_N = 128

# packed weight layouts (bytes per partition):
# w1b: w1 fp16 [256] bytes 0:512 ; b1 f32 [2] bytes 512:520
W1B_BYTES = 520
# w23: w2 fp16 [2,256] bytes 0:1024 ; w3 fp16 [2,18] bytes 1024:1096 ;
#      b2 f32 [2] bytes 1096:1104
W23_BYTES = 1104


def _halves(nb):
    out = []
    h = 0
    while h < nb:
        w = min(512, nb - h)
        out.append((h, w))
        h += w
    return out


def _chunk_hi(st, nb):
    """Index of the last xt chunk overlapping columns [st, st+nb)."""
    hi = 0
    for ci, (cst, cnb) in enumerate(XCHUNKS):
        if cst < st + nb:
            hi = ci
    return hi


def _build_v3():
    import concourse.bass as bass
    import concourse.bacc as bacc
    import concourse.mybir as mybir

    f32 = mybir.dt.float32
    f16 = mybir.dt.float16
    AF = mybir.ActivationFunctionType
    ALU = mybir.AluOpType

    nc = bacc.Bacc(None, target_bir_lowering=False, debug=False)
    xt = nc.declare_dram_parameter("xt", [OPC, OBS, PAD], f16, isOutput=False)
    w1b = nc.declare_dram_parameter("w1b", [OPC, 128, W1B_BYTES], mybir.dt.uint8,
                                    isOutput=False)
    w23 = nc.declare_dram_parameter("w23", [OPC, 128, W23_BYTES], mybir.dt.uint8,
                                    isOutput=False)
    out = nc.declare_dram_parameter("out", [OPC, A, PAD], f16, isOutput=True)

    # --- on-chip tensors ---
    xts = [nc.alloc_sbuf_tensor(f"xts{o}", [OBS, PAD], f16) for o in range(OPC)]
    w1s = [nc.alloc_sbuf_tensor(f"w1s{o}", [128, W1B_BYTES], mybir.dt.uint8)
           for o in range(OPC)]
    w23s = [nc.alloc_sbuf_tensor(f"w23s{o}", [128, W23_BYTES], mybir.dt.uint8)
            for o in range(OPC)]
    h1s = [[nc.alloc_sbuf_tensor(f"h1_{o}_{c}", [128, PAD], f16) for c in range(2)]
           for o in range(OPC)]
    h2s = [[nc.alloc_sbuf_tensor(f"h2_{o}_{m}", [128, PAD], f16) for m in range(2)]
           for o in range(OPC)]
    osbs = [nc.alloc_sbuf_tensor(f"osb{o}", [A, PAD], f16) for o in range(OPC)]
    dummy = nc.alloc_sbuf_tensor("warm_dummy", [128, WARM_N], f16)
    dummy_o = nc.alloc_sbuf_tensor("warm_dummy_o", [128, 1], f32)

    pss = [nc.alloc_psum_tensor(f"ps{s}", [128, 1024], f32) for s in range(4)]

    def w1_ap(o, c):
        return w1s[o].ap()[:, 0:512].bitcast(f16)[:, c * 128:(c + 1) * 128]

    def b1_ap(o, c):
        return w1s[o].ap()[:, 512:520].bitcast(f32)[:, c:c + 1]

    def w2_ap(o, k, m):
        base = k * 256 + m * 128
        return w23s[o].ap()[:, 0:1024].bitcast(f16)[:, base:base + 128]

    def w3_ap(o, k):
        return w23s[o].ap()[:, 1024:1096].bitcast(f16)[:, k * 18:(k + 1) * 18]

    def b2_ap(o, m):
        return w23s[o].ap()[:, 1096:1104].bitcast(f32)[:, m:m + 1]

    # --- semaphores ---
    ws = nc.alloc_semaphore("warm_sem")
    xsem = [[nc.alloc_semaphore(f"x{o}_{ci}") for ci in range(len(XCHUNKS))]
            for o in range(OPC)]
    w1sem = [nc.alloc_semaphore(f"w1t{o}") for o in range(OPC)]
    w23sem = [nc.alloc_semaphore(f"w23t{o}") for o in range(OPC)]
    fd = [nc.alloc_semaphore(f"fd{s}") for s in range(4)]
    prog = {}
    for o in range(OPC):
        for key in ("h1a", "h1v", "h2a", "h2v", "oa", "ov"):
            prog[(key, o)] = nc.alloc_semaphore(f"{key}{o}")
    odsem = nc.alloc_semaphore("od")

    # --- static schedule containers ---
    pe_ops = []
    act_ops = []
    dve_ops = []
    sync_ops = []
    gps_ops = []

    fill_count = [0, 0, 0, 0]
    slot_prev_drain = [None, None, None, None]
    fill_idx = 0
    prog_count = {k: 0 for k in prog}
    od_count = [0]

    pe_last_wait = {}

    def pe_wait(waits, sem, val):
        key = sem.name if hasattr(sem, "name") else id(sem)
        if pe_last_wait.get(key, -1) < val:
            waits.append((sem, val))
            pe_last_wait[key] = val

    def emit_fill(data_waits, mms, out_part, drains):
        """mms: (h, w, lhs_fn, rhs_fn, start, stop, mm_waits, tile_pos, ps_lo)
        drains: list of (engine, kind, dst_fn, bias_fn, psem_key, src_lo, src_hi, h0, w0)
        """
        nonlocal fill_idx
        s = fill_idx % 4
        fill_idx += 1
        waits = []
        if slot_prev_drain[s] is not None:
            for sem, cnt in slot_prev_drain[s]:
                pe_wait(waits, sem, cnt)
        for sem, val in data_waits:
            pe_wait(waits, sem, val)
        pe_ops.append((waits, s, mms, out_part, fd[s]))
        fill_count[s] += 1
        fd_thresh = fill_count[s]
        newprev = []
        for (drain_engine, kind, dst_fn, bias_fn, psem_key, src_lo, src_hi,
             h0, w0) in drains:
            sem = prog[psem_key]
            prog_count[psem_key] += 1
            cnt = prog_count[psem_key]
            op = ([(fd[s], fd_thresh)], kind, s, dst_fn, bias_fn, sem,
                  src_lo, src_hi, h0, w0)
            if drain_engine == "act":
                act_ops.append(op)
            else:
                dve_ops.append(op)
            newprev.append((sem, cnt))
        slot_prev_drain[s] = newprev
        return {k: prog_count[k] for k in prog}

    # --- input DMA schedule ---
    def xdma(o, ci):
        cst, cnb = XCHUNKS[ci]
        return ("dma", [],
                (lambda o=o, cst=cst, cnb=cnb: xts[o].ap()[:, cst:cst + cnb]),
                (lambda o=o, cst=cst, cnb=cnb: xt[o][:, cst:cst + cnb]),
                xsem[o][ci], 16)

    def wdma(o, which):
        if which == 0:
            return ("dma", [], (lambda o=o: w1s[o].ap()[:]),
                    (lambda o=o: w1b[o]), w1sem[o], 16)
        return ("dma", [], (lambda o=o: w23s[o].ap()[:]),
                (lambda o=o: w23[o]), w23sem[o], 16)

    # Three parallel issue queues, ordered by need-time. Measured cadence is
    # ~1.2-1.7us per DMA per queue, so priority order decides stalls.
    scalar_ops = [xdma(0, 0), xdma(0, 2)]
    sync_ops.extend([wdma(0, 0), xdma(0, 1), wdma(0, 1),
                     wdma(1, 0), wdma(1, 1)])
    gps_ops.extend([xdma(0, 3), xdma(0, 4), xdma(1, 0), xdma(1, 1),
                    xdma(1, 2), xdma(1, 3), xdma(1, 4)])

    di = 0
    l1_thr = {}
    l2_thr = {}

    def emit_l1(o, p):
        nonlocal di
        st, nb = PAIRS[p]
        mms = []
        drains = []
        for c in range(2):
            for h, w in _halves(nb):
                hi = _chunk_hi(st + h, w)
                mms.append((
                    h, w,
                    (lambda o=o, c=c: w1_ap(o, c)),
                    (lambda o=o, st=st, h=h, w=w: xts[o].ap()[:, st + h:st + h + w]),
                    True, True, [(xsem[o][hi], 16)], None, 0,
                ))
            eng = "act" if di % 2 == 0 else "dve"
            drains.append((eng, "relu",
                           (lambda o=o, c=c, st=st, nb=nb: h1s[o][c].ap()[:, st:st + nb]),
                           (lambda o=o, c=c: b1_ap(o, c)),
                           ("h1a" if eng == "act" else "h1v", o), 0, 128, 0, nb))
            di += 1
            # both c-chunks of one pair share a psum slot? no: separate fills
            pc = emit_fill([(w1sem[o], 16)], mms, 128, drains)
            mms = []
            drains = []
        l1_thr[(o, p)] = (pc[("h1a", o)], pc[("h1v", o)])

    def emit_l2(o, p):
        nonlocal di
        st, nb = PAIRS[p]
        na, nv = l1_thr[(o, p)]
        pc = None
        for m in range(2):
            data_waits = [(w23sem[o], 16)]
            if na:
                data_waits.append((prog[("h1a", o)], na))
            if nv:
                data_waits.append((prog[("h1v", o)], nv))
            mms = []
            for h, w in _halves(nb):
                for k in range(2):
                    mms.append((
                        h, w,
                        (lambda o=o, k=k, m=m: w2_ap(o, k, m)),
                        (lambda o=o, k=k, st=st, h=h, w=w: h1s[o][k].ap()[:, st + h:st + h + w]),
                        k == 0, k == 1, None, None, 0,
                    ))
            eng = "act" if di % 2 == 0 else "dve"
            drains = [(eng, "relu",
                       (lambda o=o, m=m, st=st, nb=nb: h2s[o][m].ap()[:, st:st + nb]),
                       (lambda o=o, m=m: b2_ap(o, m)),
                       ("h2a" if eng == "act" else "h2v", o), 0, 128, 0, nb)]
            di += 1
            pc = emit_fill(data_waits, mms, 128, drains)
        l2_thr[(o, p)] = (pc[("h2a", o)], pc[("h2v", o)])

    def emit_l3(o, p):
        nonlocal di
        st, nb = PAIRS[p]
        na, nv = l2_thr[(o, p)]
        data_waits = [(w23sem[o], 16)]
        if na:
            data_waits.append((prog[("h2a", o)], na))
        if nv:
            data_waits.append((prog[("h2v", o)], nv))
        mms = []
        for h, w in _halves(nb):
            for k in range(2):
                mms.append((
                    h, w,
                    (lambda o=o, k=k: w3_ap(o, k)),
                    (lambda o=o, k=k, st=st, h=h, w=w: h2s[o][k].ap()[:, st + h:st + h + w]),
                    k == 0, k == 1, None, None, 0,
                ))
        eng = "act" if (nb < 1024 or di % 2 == 0) else "dve"
        drains = [(eng, "copy",
                   (lambda o=o, st=st, nb=nb: osbs[o].ap()[:, st:st + nb]),
                   None,
                   ("oa" if eng == "act" else "ov", o), 0, A, 0, nb)]
        di += 1
        pc = emit_fill(data_waits, mms, A, drains)
        dma_waits = [(prog[("oa" if eng == "act" else "ov", o)],
                      pc[("oa" if eng == "act" else "ov", o)])]
        od_count[0] += 1
        dma_op = ("dma", dma_waits,
                  (lambda o=o, st=st, nb=nb: out[o][:, st:st + nb]),
                  (lambda o=o, st=st, nb=nb: osbs[o].ap()[:, st:st + nb]),
                  odsem, 16)
        sync_ops.append(dma_op)

    # global software pipeline: L1 two pair-groups ahead; L3 trails by one
    l1q = [(o, p) for o in range(OPC) for p in range(len(PAIRS))]
    l2q = list(l1q)
    l3q = []
    emit_l1(*l1q.pop(0))
    emit_l1(*l1q.pop(0))
    for (o, p) in l2q:
        emit_l2(o, p)
        if l1q:
            emit_l1(*l1q.pop(0))
        l3q.append((o, p))
        if len(l3q) > 1:
            emit_l3(*l3q.pop(0))
    while l3q:
        emit_l3(*l3q.pop(0))

    n_od = od_count[0]

    # --- emit engine programs ---
    with nc.Block() as block:

        @block.gpsimd
        def _(eng):
            nc.gpsimd.memset(dummy.ap()[:], 0.0).then_inc(ws, 1)
            for op in gps_ops:
                kind, waits, dst_fn, src_fn, sem, val = op
                for wsem_, wval in waits:
                    eng.wait_ge(wsem_, wval)
                eng.dma_start(out=dst_fn(), in_=src_fn()).then_inc(sem, val)

        @block.sync
        def _(eng):
            for op in sync_ops:
                kind, waits, dst_fn, src_fn, sem, val = op
                for wsem_, wval in waits:
                    eng.wait_ge(wsem_, wval)
                eng.dma_start(out=dst_fn(), in_=src_fn()).then_inc(sem, val)
            eng.wait_ge(odsem, 16 * n_od)

        @block.tensor
        def _(eng):
            eng.wait_ge(ws, 1)
            for _i in range(N_WARM):
                nc.tensor.matmul(
                    pss[0].ap()[:128, :WARM_N], dummy.ap()[:, :], dummy.ap()[:, :],
                    start=True, stop=True,
                )
            mm_seen = {}
            for waits, s, mms, out_part, fdsem in pe_ops:
                for wsem_, wval in waits:
                    eng.wait_ge(wsem_, wval)
                for j, (h, w, lhs_fn, rhs_fn, stt, stp, mwaits, tp, plo) in enumerate(mms):
                    if mwaits:
                        for wsem_, wval in mwaits:
                            key = wsem_.name if hasattr(wsem_, "name") else id(wsem_)
                            if mm_seen.get(key, -1) < wval:
                                eng.wait_ge(wsem_, wval)
                                mm_seen[key] = wval
                    kw = {}
                    if tp is not None:
                        kw["tile_position"] = tp
                    inst = nc.tensor.matmul(
                        pss[s].ap()[plo:plo + out_part, h:h + w],
                        lhs_fn(), rhs_fn(), start=stt, stop=stp, **kw,
                    )
                    if j == len(mms) - 1:
                        inst.then_inc(fdsem, 1)

        @block.scalar
        def _(eng):
            for op in scalar_ops:
                kind, waits, dst_fn, src_fn, sem, val = op
                eng.dma_start(out=dst_fn(), in_=src_fn()).then_inc(sem, val)
            eng.wait_ge(ws, 1)
            nc.scalar.activation(dummy_o.ap()[:], dummy.ap()[:, 0:1], AF.Relu, bias=0.0)
            for waits, kind, s, dst_fn, bias_fn, sem, lo, hi, h0, w0 in act_ops:
                for wsem_, wval in waits:
                    eng.wait_ge(wsem_, wval)
                dst = dst_fn()
                src = pss[s].ap()[lo:hi, h0:h0 + w0]
                if kind == "relu":
                    inst = nc.scalar.activation(dst, src, AF.Relu, bias=bias_fn())
                else:
                    inst = nc.scalar.activation(dst, src, AF.Copy)
                inst.then_inc(sem, 1)

        @block.vector
        def _(eng):
            for waits, kind, s, dst_fn, bias_fn, sem, lo, hi, h0, w0 in dve_ops:
                for wsem_, wval in waits:
                    eng.wait_ge(wsem_, wval)
                dst = dst_fn()
                src = pss[s].ap()[lo:hi, h0:h0 + w0]
                if kind == "relu":
                    inst = nc.vector.tensor_scalar(
                        dst, src, bias_fn(), 0.0, ALU.add, ALU.max
                    )
                else:
                    inst = nc.vector.tensor_copy(dst, src)
                inst.then_inc(sem, 1)

    nc.compile()
    return nc


def _get_program():
    if "nc" not in _CACHE:
        _CACHE["nc"] = _build_v3()
    return _CACHE["nc"]


def _prep(inputs):
    obs = np.ascontiguousarray(np.asarray(inputs["obs"], dtype=np.float32))
    option = np.asarray(inputs["option"]).astype(np.int64, copy=False)
    W1 = np.asarray(inputs["W1"], dtype=np.float32)
    b1 = np.asarray(inputs["b1"], dtype=np.float32)
    W2 = np.asarray(inputs["W2"], dtype=np.float32)
    b2 = np.asarray(inputs["b2"], dtype=np.float32)
    W3 = np.asarray(inputs["W3"], dtype=np.float32)
    b3 = np.asarray(inputs["b3"], dtype=np.float32)

    order = np.argsort(option, kind="stable")
    sorted_opt = option[order]
    starts = np.searchsorted(sorted_opt, np.arange(OPT + 1))
    idx_per_opt = [order[starts[o]: starts[o + 1]] for o in range(OPT)]

    in_maps = []
    for core in range(NCORES):
        xtc = np.zeros((OPC, OBS, PAD), np.float16)
        w1bc = np.zeros((OPC, 128, W1B_BYTES), np.uint8)
        w23c = np.zeros((OPC, 128, W23_BYTES), np.uint8)
        for lo in range(OPC):
            o = core * OPC + lo
            idx = idx_per_opt[o][:PAD]
            xtc[lo, :, : len(idx)] = obs[idx].T
            w1p = np.ascontiguousarray(W1[o].astype(np.float16))  # [128, 256]
            b1p = np.ascontiguousarray(b1[o].reshape(2, 128).T.astype(np.float32))
            w1bc[lo] = np.concatenate(
                [w1p.view(np.uint8), b1p.view(np.uint8)], axis=1)
            w2p = np.ascontiguousarray(
                W2[o].reshape(2, 128, H2).transpose(1, 0, 2).astype(np.float16)
            ).reshape(128, -1)                                    # [128, 512]
            w3p = np.ascontiguousarray(
                W3[o].reshape(2, 128, A).transpose(1, 0, 2).astype(np.float16)
            ).reshape(128, -1)                                    # [128, 36]
            b2p = np.ascontiguousarray(b2[o].reshape(2, 128).T.astype(np.float32))
            w23c[lo] = np.concatenate(
                [w2p.view(np.uint8), w3p.view(np.uint8), b2p.view(np.uint8)],
                axis=1)
        in_maps.append({"xt": xtc, "w1b": w1bc, "w23": w23c})
    host = dict(obs=obs, W1=W1, b1=b1, W2=W2, b2=b2, W3=W3, b3=b3)
    return in_maps, idx_per_opt, host


def _unshard(results, idx_per_opt, host):
    out_full = np.empty((B, 1, A), np.float32)
    for core in range(NCORES):
        res = results[core]["out"]  # [OPC, A, PAD]
        for lo in range(OPC):
            o = core * OPC + lo
            idx = idx_per_opt[o]
            n = min(len(idx), PAD)
            out_full[idx[:n], 0, :] = res[lo, :, :n].T + host["b3"][o]
            if len(idx) > n:  # overflow beyond PAD: compute on host (rare/never)
                rows = host["obs"][idx[n:]]
                h = np.maximum(rows @ host["W1"][o] + host["b1"][o], 0.0)
                h = np.maximum(h @ host["W2"][o] + host["b2"][o], 0.0)
                out_full[idx[n:], 0, :] = h @ host["W3"][o] + host["b3"][o]
    return out_full


def run(inputs, trace=False, **spmd_kwargs):
    """Run the kernel; returns (output, BassKernelResults)."""
    from concourse.bass_utils import run_bass_kernel_spmd

    in_maps, idx_per_opt, host = _prep(inputs)
    nc = _get_program()
    try:
        br = run_bass_kernel_spmd(
            nc, in_maps, list(range(NCORES)), trace=trace, **spmd_kwargs
        )
    except Exception:
        _CACHE.clear()
        nc = _get_program()
        br = run_bass_kernel_spmd(
            nc, in_maps, list(range(NCORES)), trace=trace, **spmd_kwargs
        )
    return _unshard(br.results, idx_per_opt, host), br


def kernel(**inputs):
    out, _ = run(inputs)
    return out
